# revision 2
# baseline (speedup 1.0000x reference)
"""Bass/Trainium2 SPMD kernel for EnhancedMultiScaleAdaptiveElasticityLossWithLame.

Strategy: shard the X spatial axis across 8 NeuronCores. Host slices inputs
(with 1-slice halos, extrapolation-padded at global edges so one-sided
boundary gradients == central diffs on the padded slab), pre-resizes the
scale-1/2 fields (trilinear align-corners, exactly matching the reference
formula), and precomputes gradient-magnitude halo slices needed by the
separable Gaussian blur's X taps. Each core computes, per scale:
  - deform gradients: X/Z via shifted-AP vector ops, Y via PE matmul with an
    exact banded gradient matrix (one-sided rows at the boundary),
  - image gradient magnitude, separable 5-tap Gaussian blur (Z via shifted
    adds, X via the slice ring, Y via PE matmul with the reflect-padded blur
    matrix, center-tap normalizations folded into the Y matrix),
  - energy via the identity  weight*energy = f(ig)*(trE^2 + ||E||_F^2)
    with f(t) = 5*(t+0.35)^2 - 0.1125  (the clamps provably never bind for
    image in [0,1)), accumulated with fused affine_mul_reduce ops.
Host sums the per-core/per-cell partials, adds the (18-element) Jacobian
penalty, and returns the scalar.
"""

import numpy as np

# ---------------- constants (hardcoded from the problem spec) --------------
B = 2
N0 = 160
NCORES = 8
SCALES = [0, 1, 2]
NS = {0: 160, 1: 80, 2: 40}          # cube side per scale
WS = {0: 20, 1: 10, 2: 5}            # interior X slices per core per scale
SW = [1.0, 0.5, 0.25]                # scale weights
JW = 0.1                             # jacobian penalty weight
BLUR_SIGMA = 1.1

_t = np.arange(5, dtype=np.float64) - 2.0
_k = np.exp(-(_t ** 2) / (2.0 * BLUR_SIGMA ** 2))
_k = _k / _k.sum()                   # [a, b, c, b, a]
KA, KB, KC = float(_k[0]), float(_k[1]), float(_k[2])


# ---------------- host-side numerics (match reference exactly) -------------
def _rz1(x, axis, out):
    n = x.shape[axis]
    if out == n:
        return x
    coords = np.arange(out, dtype=np.float32) * np.float32((n - 1) / max(out - 1, 1))
    i0 = np.floor(coords).astype(np.int32)
    i1 = np.minimum(i0 + 1, n - 1)
    w = (coords - i0.astype(np.float32)).astype(np.float32)
    shape = [1] * x.ndim
    shape[axis] = out
    w = w.reshape(shape)
    x0 = np.take(x, i0, axis=axis)
    x1 = np.take(x, i1, axis=axis)
    return (x0 * (1 - w) + x1 * w).astype(np.float32)


def _resize3d(x, s):
    # x [..., X, Y, Z] -> all three trailing axes resized to s (X,Y,Z order)
    for ax in (-3, -2, -1):
        x = _rz1(x, ax % x.ndim, s)
    return x


def _mag(img):
    # img [B, X, Y, Z] -> gradient magnitude, float32
    gx, gy, gz = np.gradient(img.astype(np.float32), axis=(1, 2, 3))
    return np.sqrt(gx * gx + gy * gy + gz * gz).astype(np.float32)


def _jac_penalty(d):
    Bn, _, X, Y, Z = d.shape
    c = (X // 2, Y // 2, Z // 2)
    dx = 0.5 * (d[:, :, c[0] + 1, c[1], c[2]] - d[:, :, c[0] - 1, c[1], c[2]])
    dy = 0.5 * (d[:, :, c[0], c[1] + 1, c[2]] - d[:, :, c[0], c[1] - 1, c[2]])
    dz = 0.5 * (d[:, :, c[0], c[1], c[2] + 1] - d[:, :, c[0], c[1], c[2] - 1])
    J = np.stack([dx, dy, dz], axis=-1)  # (B,3,3)
    det = np.linalg.det(J.astype(np.float64))
    return float(np.mean(np.maximum(-det, 0.0)))


def _slab(full, lo, hi):
    """full [B, C, X, n, n] -> x slices [lo, hi) with linear-extrap at OOB."""
    X = full.shape[2]
    idx = np.clip(np.arange(lo, hi), 0, X - 1)
    sl = full[:, :, idx].astype(np.float32).copy()
    if lo < 0:
        sl[:, :, 0] = 2.0 * full[:, :, 0] - full[:, :, 1]
    if hi > X:
        sl[:, :, -1] = 2.0 * full[:, :, -1] - full[:, :, -2]
    return sl


def _mag_halo(magf, lo, W):
    """magf [B, X, n, n] -> slices at [lo-2, lo-1, lo+W, lo+W+1], reflected."""
    X = magf.shape[1]
    pos = []
    for p in (lo - 2, lo - 1, lo + W, lo + W + 1):
        if p < 0:
            p = -p
        elif p >= X:
            p = 2 * (X - 1) - p
        pos.append(p)
    return magf[:, pos]


# ---------------- operator matrices ---------------------------------------
def _grad_matrix(n):
    G = np.zeros((n, n), dtype=np.float64)
    for i in range(1, n - 1):
        G[i, i - 1], G[i, i + 1] = -0.5, 0.5
    G[0, 0], G[0, 1] = -1.0, 1.0
    G[n - 1, n - 2], G[n - 1, n - 1] = -1.0, 1.0
    return G


def _blur_matrix(n):
    Bm = np.zeros((n, n), dtype=np.float64)
    for i in range(n):
        for t in range(5):
            j = i + t - 2
            if j < 0:
                j = -j
            elif j >= n:
                j = 2 * (n - 1) - j
            Bm[i, j] += _k[t]
    return Bm * (KC * KC)  # fold the Z and X center-tap normalizations


# ---------------- device kernel build -------------------------------------
_CACHE = {}


def _build_nc():
    import concourse.bacc as bacc
    import concourse.mybir as mybir
    from concourse.tile import TileContext

    ALU = mybir.AluOpType
    AF = mybir.ActivationFunctionType
    F32 = mybir.dt.float32
    R1, R2 = KB / KC, KA / KC

    nc = bacc.Bacc("TRN2", target_bir_lowering=False, debug=False,
                   num_devices=NCORES)

    dram = {}
    for s in SCALES:
        n, S = NS[s], WS[s] + 2
        dram[f"d{s}"] = nc.dram_tensor(f"d{s}", (B, n, 3, S, n), F32,
                                       kind="ExternalInput")
        dram[f"i{s}"] = nc.dram_tensor(f"i{s}", (B, n, S, n), F32,
                                       kind="ExternalInput")
        dram[f"m{s}"] = nc.dram_tensor(f"m{s}", (B, n, 4, n), F32,
                                       kind="ExternalInput")
    acc_out = nc.dram_tensor("acc", (128, 128), F32, kind="ExternalOutput")

    # inline constant matrices (transposed: lhsT[k=y_in, m=y_out])
    consts = {}
    for s in SCALES:
        n = NS[s]
        GT = np.ascontiguousarray(_grad_matrix(n).T).astype(np.float32)
        BT = np.ascontiguousarray(_blur_matrix(n).T).astype(np.float32)
        if s == 0:
            for nm, M in (("g", GT), ("y", BT)):
                consts[f"{nm}0_00"] = nc.inline_tensor(
                    np.ascontiguousarray(M[0:128, 0:128]), name=f"{nm}0_00")
                consts[f"{nm}0_10"] = nc.inline_tensor(
                    np.ascontiguousarray(M[128:160, 0:128]), name=f"{nm}0_10")
                consts[f"{nm}0_01"] = nc.inline_tensor(
                    np.ascontiguousarray(M[0:128, 128:160]), name=f"{nm}0_01")
                consts[f"{nm}0_11"] = nc.inline_tensor(
                    np.ascontiguousarray(M[128:160, 128:160]), name=f"{nm}0_11")
        else:
            consts[f"g{s}"] = nc.inline_tensor(GT, name=f"g{s}")
            consts[f"y{s}"] = nc.inline_tensor(BT, name=f"y{s}")

    with TileContext(nc) as tc:
        with tc.tile_pool(name="mats", bufs=1) as matp, \
             tc.tile_pool(name="slab", bufs=1) as slabp, \
             tc.tile_pool(name="work", bufs=1) as wp, \
             tc.tile_pool(name="ring", bufs=6) as ringp, \
             tc.tile_pool(name="accp", bufs=1) as accp, \
             tc.tile_pool(name="ps", bufs=1, space="PSUM") as psp:

            # --- load matrices into SBUF
            mt = {}
            for key, h in consts.items():
                t = matp.tile(list(h.shape), F32, name=f"mt_{key}")
                nc.sync.dma_start(out=t[:], in_=h[:])
                mt[key] = t

            b035 = matp.tile([128, 1], F32, name="b035")
            nc.vector.memset(b035[:], 0.35)

            acc_t = accp.tile([128, 128], F32, name="acc_t")
            nc.vector.memset(acc_t[:], 0.0)
            col = [0]

            def tt(out, a, bb, op):
                nc.vector.tensor_tensor(out=out, in0=a, in1=bb, op=op)

            def stt(out, a, sc, bb):
                nc.vector.scalar_tensor_tensor(
                    out=out, in0=a, scalar=sc, in1=bb,
                    op0=ALU.mult, op1=ALU.add)

            def blur5(P, n, center, m1, p1, m2, p2, outt):
                t2 = wp.tile([P, n], F32, name="bl_t2")
                nc.gpsimd.tensor_tensor(out=t2[:], in0=m1, in1=p1, op=ALU.add)
                t1 = wp.tile([P, n], F32, name="bl_t1")
                nc.gpsimd.tensor_tensor(out=t1[:], in0=m2, in1=p2, op=ALU.add)
                sB = wp.tile([P, n], F32, name="bl_sB")
                stt(sB[:], t2[:], R1, center)
                stt(outt, t1[:], R2, sB[:])

            # --------------- per scale ---------------
            XB = 3
            scale_ranges = {}
            for s in SCALES:
                n, W, S = NS[s], WS[s], WS[s] + 2
                col_start = col[0]
                if s == 0:
                    chunks = [
                        dict(P=128, ysl=slice(0, 128), gm=("00", "10"),
                             ym=("00", "10")),
                        dict(P=32, ysl=slice(128, 160), gm=("01", "11"),
                             ym=("01", "11")),
                    ]
                else:
                    chunks = [dict(P=n, ysl=slice(0, n), gm=None, ym=None)]
                groups = []
                g0 = 0
                while g0 < W:
                    groups.append((g0, min(g0 + XB, W) - 1))
                    g0 += XB

                for b in range(B):
                    for ci, ch in enumerate(chunks):
                        dmt = slabp.tile([ch["P"], 3, S, n], F32,
                                         name=f"dm_{s}_{ci}")
                        nc.sync.dma_start(out=dmt[:],
                                          in_=dram[f"d{s}"][b, ch["ysl"]])
                        imt = slabp.tile([ch["P"], S, n], F32,
                                         name=f"im_{s}_{ci}")
                        nc.sync.dma_start(out=imt[:],
                                          in_=dram[f"i{s}"][b, ch["ysl"]])
                        mht = slabp.tile([ch["P"], 4, n], F32,
                                         name=f"mh_{s}_{ci}")
                        nc.sync.dma_start(out=mht[:],
                                          in_=dram[f"m{s}"][b, ch["ysl"]])
                        ch["dm"], ch["im"], ch["mh"] = dmt, imt, mht

                    def mm_into(psum_t, suffix_pair, kind, rhs_of):
                        if suffix_pair is None:
                            w = mt[f"{kind}{s}"]
                            nc.tensor.matmul(psum_t, w[:], rhs_of(chunks[0]),
                                             start=True, stop=True)
                        else:
                            for j, suf in enumerate(suffix_pair):
                                w = mt[f"{kind}0_{suf}"]
                                nc.tensor.matmul(psum_t, w[:],
                                                 rhs_of(chunks[j]),
                                                 start=(j == 0),
                                                 stop=(j == len(suffix_pair) - 1))

                    P1rings = [dict() for _ in chunks]
                    p2gs = [None for _ in chunks]

                    for x in range(-2, W + 2):
                        t0 = x - 2
                        for ci, ch in enumerate(chunks):
                            P = ch["P"]
                            im, dm, mh = ch["im"], ch["dm"], ch["mh"]
                            # ---- mag[x]
                            if 0 <= x < W:
                                igy = psp.tile([P, n], F32, name=f"igy_{ci}", bufs=2 if ci == 0 else 1)
                                mm_into(igy[:], ch["gm"], "g",
                                        lambda c: c["im"][:, x + 1, :])
                                gxr = wp.tile([P, n], F32, name="gxr")
                                tt(gxr[:], im[:, x + 2, :], im[:, x, :],
                                   ALU.subtract)
                                gzr = wp.tile([P, n], F32, name="gzr")
                                tt(gzr[:, 1:n - 1], im[:, x + 1, 2:n],
                                   im[:, x + 1, 0:n - 2], ALU.subtract)
                                tt(gzr[:, 0:n:n - 1],
                                   im[:, x + 1, 1:n:n - 2],
                                   im[:, x + 1, 0:n - 1:n - 2], ALU.subtract)
                                nc.vector.tensor_scalar_mul(
                                    gzr[:, 0:n:n - 1], gzr[:, 0:n:n - 1], 2.0)
                                q1 = wp.tile([P, n], F32, name="q1")
                                nc.scalar.activation(q1[:], gxr[:], AF.Square,
                                                     scale=0.5)
                                q2 = wp.tile([P, n], F32, name="q2")
                                nc.scalar.activation(q2[:], gzr[:], AF.Square,
                                                     scale=0.5)
                                q3 = wp.tile([P, n], F32, name="q3")
                                nc.scalar.activation(q3[:], igy[:], AF.Square)
                                s12 = wp.tile([P, n], F32, name="s12")
                                nc.gpsimd.tensor_tensor(out=s12[:], in0=q1[:],
                                                        in1=q2[:], op=ALU.add)
                                m2t = wp.tile([P, n], F32, name="m2t")
                                tt(m2t[:], s12[:], q3[:], ALU.add)
                                magt = wp.tile([P, n], F32, name="magt")
                                nc.scalar.activation(magt[:], m2t[:], AF.Sqrt)
                                mag_ap = magt[:]
                            else:
                                hidx = x + 2 if x < 0 else x - W + 2
                                mag_ap = mh[:, hidx, :]
                            # ---- P1[x] = blur_z(mag)/KC
                            pm = wp.tile([P, n + 4], F32, name="pm")
                            nc.gpsimd.tensor_copy(out=pm[:, 2:n + 2],
                                                  in_=mag_ap)
                            nc.gpsimd.tensor_copy(out=pm[:, 0:1],
                                                  in_=pm[:, 4:5])
                            nc.gpsimd.tensor_copy(out=pm[:, 1:2],
                                                  in_=pm[:, 3:4])
                            nc.gpsimd.tensor_copy(out=pm[:, n + 2:n + 3],
                                                  in_=pm[:, n:n + 1])
                            nc.gpsimd.tensor_copy(out=pm[:, n + 3:n + 4],
                                                  in_=pm[:, n - 1:n])
                            p1t = ringp.tile([P, n], F32, name=f"p1r_{ci}")
                            blur5(P, n, pm[:, 2:n + 2], pm[:, 1:n + 1],
                                  pm[:, 3:n + 3], pm[:, 0:n], pm[:, 4:n + 4],
                                  p1t[:])
                            P1rings[ci][x] = p1t

                        if not (0 <= t0 < W):
                            continue
                        gi = t0 // XB
                        g0, g1 = groups[gi]
                        xb = g1 - g0 + 1
                        for ci, ch in enumerate(chunks):
                            P = ch["P"]
                            rg = P1rings[ci]
                            if t0 == g0:
                                p2gs[ci] = wp.tile([P, XB, n], F32,
                                                   name=f"p2g_{ci}")
                            blur5(P, n, rg[t0][:], rg[t0 - 1][:],
                                  rg[t0 + 1][:], rg[t0 - 2][:],
                                  rg[t0 + 2][:], p2gs[ci][:, t0 - g0, :])
                        if t0 != g1:
                            continue
                        # ---- grouped energy block
                        for ci, ch in enumerate(chunks):
                            P = ch["P"]
                            dm = ch["dm"]
                            ig = psp.tile([P, xb, n], F32, name="ig", bufs=2)
                            mm_into(ig[:], ch["ym"], "y",
                                    lambda c: p2gs[chunks.index(c)][:, 0:xb, :])
                            u = wp.tile([P, xb, n], F32, name="u")
                            nc.scalar.activation(u[:], ig[:], AF.Square,
                                                 bias=b035[0:P, :])
                            yps = []
                            for c_i in range(3):
                                ypc = psp.tile([P, xb, n], F32,
                                               name=f"ypc{c_i}")
                                mm_into(ypc[:], ch["gm"], "g",
                                        lambda c, c_i=c_i:
                                        c["dm"][:, c_i, g0 + 1:g0 + 1 + xb, :])
                                yps.append(ypc)
                            dgx = wp.tile([P, 3, xb, n], F32, name="dgx")
                            tt(dgx[:], dm[:, :, g0 + 2:g0 + 2 + xb, :],
                               dm[:, :, g0:g0 + xb, :], ALU.subtract)
                            dgz = wp.tile([P, 3, xb, n], F32, name="dgz")
                            tt(dgz[:, :, :, 1:n - 1],
                               dm[:, :, g0 + 1:g0 + 1 + xb, 2:n],
                               dm[:, :, g0 + 1:g0 + 1 + xb, 0:n - 2],
                               ALU.subtract)
                            tt(dgz[:, :, :, 0:n:n - 1],
                               dm[:, :, g0 + 1:g0 + 1 + xb, 1:n:n - 2],
                               dm[:, :, g0 + 1:g0 + 1 + xb, 0:n - 1:n - 2],
                               ALU.subtract)
                            nc.vector.tensor_scalar_mul(
                                dgz[:, :, :, 0:n:n - 1],
                                dgz[:, :, :, 0:n:n - 1], 2.0)
                            s1 = wp.tile([P, xb, n], F32, name="s1")
                            stt(s1[:], dgx[:, 0], 0.5, yps[1][:])
                            trE = wp.tile([P, xb, n], F32, name="trE")
                            stt(trE[:], dgz[:, 2], 0.5, s1[:])
                            p4 = wp.tile([P, xb, n], F32, name="p4")
                            stt(p4[:], dgx[:, 1], 0.5, yps[0][:])
                            p6 = wp.tile([P, xb, n], F32, name="p6")
                            stt(p6[:], dgz[:, 1], 0.5, yps[2][:])
                            p5 = wp.tile([P, xb, n], F32, name="p5")
                            nc.gpsimd.tensor_tensor(out=p5[:], in0=dgz[:, 0],
                                                    in1=dgx[:, 2], op=ALU.add)
                            S1 = wp.tile([P, xb, n], F32, name="S1")
                            tt(S1[:], trE[:], trE[:], ALU.mult)
                            S2 = wp.tile([P, xb, n], F32, name="S2")
                            tt(S2[:], p4[:], p4[:], ALU.mult)
                            S3 = wp.tile([P, xb, n], F32, name="S3")
                            nc.gpsimd.tensor_tensor(out=S3[:], in0=p5[:],
                                                    in1=p5[:], op=ALU.mult)
                            S4 = wp.tile([P, xb, n], F32, name="S4")
                            tt(S4[:], p6[:], p6[:], ALU.mult)
                            S5 = wp.tile([P, xb, n], F32, name="S5")
                            nc.scalar.activation(S5[:], yps[1][:], AF.Square)
                            S6 = wp.tile([P, xb, n], F32, name="S6")
                            nc.scalar.activation(S6[:], dgx[:, 0], AF.Square)
                            S7 = wp.tile([P, xb, n], F32, name="S7")
                            nc.scalar.activation(S7[:], dgz[:, 2], AF.Square)
                            r1 = wp.tile([P, xb, n], F32, name="r1")
                            stt(r1[:], S6[:], 0.25, S1[:])
                            r2 = wp.tile([P, xb, n], F32, name="r2")
                            stt(r2[:], S7[:], 0.25, S5[:])
                            r3 = wp.tile([P, xb, n], F32, name="r3")
                            stt(r3[:], S2[:], 0.5, r1[:])
                            r4 = wp.tile([P, xb, n], F32, name="r4")
                            stt(r4[:], S4[:], 0.5, r2[:])
                            r5 = wp.tile([P, xb, n], F32, name="r5")
                            stt(r5[:], S3[:], 0.125, r3[:])
                            Rt = wp.tile([P, xb, n], F32, name="Rt")
                            tt(Rt[:], r4[:], r5[:], ALU.add)
                            scr = wp.tile([P, xb, n], F32, name="scr")
                            nc.vector.affine_mul_reduce(
                                out=scr[:], accum_out=acc_t[0:P,
                                                           col[0]:col[0] + 1],
                                in0=u[:], in1=Rt[:], scale=5.0, bias=-0.1125)
                            col[0] += 1
                scale_ranges[s] = (col_start, col[0])

            nc.sync.dma_start(out=acc_out[:], in_=acc_t[:])
            _build_nc.scale_ranges = scale_ranges

    nc.compile()
    return nc, _build_nc.scale_ranges


def kernel(deformation_field, image):
    if "nc" not in _CACHE:
        _CACHE["nc"], _CACHE["ranges"] = _build_nc()
    nc, ranges = _CACHE["nc"], _CACHE["ranges"]
    from concourse.bass_utils import run_bass_kernel_spmd

    d0 = np.asarray(deformation_field, dtype=np.float32)
    i0full = np.asarray(image, dtype=np.float32)

    # host: resized fields + mags per scale
    dd = {0: d0}
    ii = {0: i0full[:, 0]}
    for s in (1, 2):
        dd[s] = _resize3d(d0, NS[s])
        ii[s] = _resize3d(i0full, NS[s])[:, 0]
    mags = {s: _mag(ii[s]) for s in SCALES}

    in_maps = []
    for k in range(NCORES):
        im = {}
        for s in SCALES:
            n, W = NS[s], WS[s]
            lo = W * k
            dsl = _slab(dd[s], lo - 1, lo + W + 1)              # [B,3,S,n,n]
            isl = _slab(ii[s][:, None], lo - 1, lo + W + 1)[:, 0]  # [B,S,n,n]
            mh = _mag_halo(mags[s], lo, W)                      # [B,4,n,n]
            im[f"d{s}"] = np.ascontiguousarray(
                dsl.transpose(0, 3, 1, 2, 4)).astype(np.float32)
            im[f"i{s}"] = np.ascontiguousarray(
                isl.transpose(0, 2, 1, 3)).astype(np.float32)
            im[f"m{s}"] = np.ascontiguousarray(
                mh.transpose(0, 2, 1, 3)).astype(np.float32)
        in_maps.append(im)

    _CACHE["in_maps"] = in_maps
    res = run_bass_kernel_spmd(nc, in_maps, core_ids=list(range(NCORES)))

    total = 0.0
    for s in SCALES:
        c0, c1 = ranges[s]
        ssum = 0.0
        for r in res.results:
            ssum += float(r["acc"][:, c0:c1].sum(dtype=np.float64))
        total += SW[s] * ssum / (B * NS[s] ** 3)
    total += JW * _jac_penalty(d0)
    return np.float32(total)

